# revision 7
# baseline (speedup 1.0000x reference)
"""MixUp1D on Trainium2 (Bass/Tile), 8-core data-parallel.

Computes, for x:(N,C,L), y:(N,NC), perm/mask/lam:(N,):
    w        = where(mask, lam, 1.0)                  (host, O(N) scalars)
    aug_x[i] = w[i]*x[i] + (1-w[i])*x[perm[i]]
    aug_y[i] = w[i]*y[i] + (1-w[i])*y[perm[i]]

Sharding: N is split across 8 cores (32 samples each). perm indexes the
whole batch, so the cross-shard gather x[perm] is materialized on the
host while building the per-core inputs (the "all-to-all" realized at
shard time). Sample->core assignment is load-balanced: masked samples
are dealt round-robin so every core gets within +-1 of the mean.

Per-core device program (one NEFF per blend count B, cached):
  - First B samples (all masked ones, masked-first order): blend.
      4-sample tiles [128, 8192] (4 MB contiguous DMAs; 32 partitions
      per sample) + B%4 single-sample tiles [128, 2048], so the
      per-sample weight is a per-partition scalar:
        x_t  *= w                 (ACT Copy, per-partition scale, in place)
        xp_t  = xp_t*(1-w) + x_t  (fused scalar_tensor_tensor, in place)
  - Remaining samples have mask=False, so out == x bit-exactly
    (w=1 blend is 1*x + 0*xp): one contiguous DRAM->DRAM copy, skipping
    their x[perm] read entirely.
"""

import numpy as np

import concourse.bacc as bacc
import concourse.mybir as mybir
from concourse.tile import TileContext
from concourse.bass_utils import run_bass_kernel_spmd

# Problem shapes (fixed for this problem)
N, C, L = 256, 16, 16384
NCLS = 8
NCORES = 8
SHARD = N // NCORES          # 32 samples per core
ELEM = C * L                 # 262144 elements per sample
F = 8192                     # big-tile free dim (4 samples / 4 MB per tile)
FS = ELEM // 128             # single-sample tile free dim (2048)
P = 128                      # SBUF partitions
SPT = (P * F) // ELEM        # samples per big tile (4)
PPS = P // SPT               # partitions per sample in a big tile (32)

_CACHE: dict = {}


def _build_nc(b: int):
    """Build + compile the SPMD program blending the first `b` samples
    (1 <= b <= SHARD) and bulk-copying the rest."""
    assert 1 <= b <= SHARD
    nf, nr = divmod(b, SPT)      # full 4-sample tiles, single-sample tiles
    ncols = nf + nr              # weight columns
    f32 = mybir.dt.float32
    nc = bacc.Bacc(
        "TRN2",
        target_bir_lowering=False,
        debug=False,
        enable_asserts=False,
        num_devices=NCORES,
    )
    xs = nc.dram_tensor("xs", [SHARD * ELEM], f32, kind="ExternalInput")
    xps = nc.dram_tensor("xps", [b * ELEM], f32, kind="ExternalInput")
    wc = nc.dram_tensor("wc", [P, ncols], f32, kind="ExternalInput")
    owc = nc.dram_tensor("owc", [P, ncols], f32, kind="ExternalInput")
    ys = nc.dram_tensor("ys", [SHARD, NCLS], f32, kind="ExternalInput")
    yps = nc.dram_tensor("yps", [SHARD, NCLS], f32, kind="ExternalInput")
    wy = nc.dram_tensor("wy", [SHARD, 2], f32, kind="ExternalInput")
    ox = nc.dram_tensor("ox", [SHARD * ELEM], f32, kind="ExternalOutput")
    oy = nc.dram_tensor("oy", [SHARD, NCLS], f32, kind="ExternalOutput")

    Copy = mybir.ActivationFunctionType.Copy
    mult = mybir.AluOpType.mult
    add = mybir.AluOpType.add

    def blk(t1d, start_elems, p, f):
        return t1d[start_elems : start_elems + p * f].rearrange(
            "(p f) -> p f", p=p
        )

    with TileContext(nc) as tc:
        with (
            tc.tile_pool(name="const", bufs=1) as cp,
            tc.tile_pool(name="io", bufs=3) as io,
        ):
            wct = cp.tile([P, ncols], f32, tag="wct")
            owct = cp.tile([P, ncols], f32, tag="owct")
            nc.sync.dma_start(out=wct[:], in_=wc[:])
            nc.sync.dma_start(out=owct[:], in_=owc[:])

            # Unmasked tail: out == x, no x[perm] read needed. One
            # contiguous DRAM->DRAM copy on the SWDGE ring.
            if b < SHARD:
                nc.gpsimd.dma_start(out=ox[b * ELEM :], in_=xs[b * ELEM :])

            # 4-sample blend tiles
            for t in range(nf):
                off = t * SPT * ELEM
                xt = io.tile([P, F], f32, tag="x")
                xpt = io.tile([P, F], f32, tag="xp")
                nc.sync.dma_start(out=xt[:], in_=blk(xs, off, P, F))
                nc.sync.dma_start(out=xpt[:], in_=blk(xps, off, P, F))
                # x_t *= w (in-place on ScalarE)
                nc.scalar.activation(xt[:], xt[:], Copy, scale=wct[:, t : t + 1])
                # xp_t = (xp_t * (1-w)) + x_t (fused, in-place on VectorE)
                nc.vector.scalar_tensor_tensor(
                    xpt[:], xpt[:], owct[:, t : t + 1], xt[:], mult, add
                )
                # store from the ACT HWDGE ring so loads (SP ring) aren't
                # head-of-line blocked behind stores
                nc.scalar.dma_start(out=blk(ox, off, P, F), in_=xpt[:])

            # single-sample boundary blend tiles
            for j in range(nr):
                off = (nf * SPT + j) * ELEM
                c = nf + j
                xt = io.tile([P, FS], f32, tag="x")
                xpt = io.tile([P, FS], f32, tag="xp")
                nc.sync.dma_start(out=xt[:], in_=blk(xs, off, P, FS))
                nc.sync.dma_start(out=xpt[:], in_=blk(xps, off, P, FS))
                nc.scalar.activation(xt[:], xt[:], Copy, scale=wct[:, c : c + 1])
                nc.vector.scalar_tensor_tensor(
                    xpt[:], xpt[:], owct[:, c : c + 1], xt[:], mult, add
                )
                nc.scalar.dma_start(out=blk(ox, off, P, FS), in_=xpt[:])

            # y path: one tiny [32, 8] tile (full blend, original order)
            wyt = cp.tile([SHARD, 2], f32, tag="wyt")
            yt = cp.tile([SHARD, NCLS], f32, tag="yt")
            ypt = cp.tile([SHARD, NCLS], f32, tag="ypt")
            nc.sync.dma_start(out=wyt[:], in_=wy[:])
            nc.sync.dma_start(out=yt[:], in_=ys[:])
            nc.sync.dma_start(out=ypt[:], in_=yps[:])
            nc.scalar.activation(yt[:], yt[:], Copy, scale=wyt[:, 0:1])
            nc.vector.scalar_tensor_tensor(
                ypt[:], ypt[:], wyt[:, 1:2], yt[:], mult, add
            )
            nc.scalar.dma_start(out=oy[:], in_=ypt[:])

    nc.compile()
    return nc


def get_nc(b: int = SHARD):
    if b not in _CACHE:
        _CACHE[b] = _build_nc(b)
    return _CACHE[b]


def _plan(mask_b):
    """Load-balanced sample->core assignment, masked-first per core.

    Returns (b, assign) where assign[k] is the array of 32 global sample
    ids for core k (its masked samples first), and b is the per-core
    blend count (max masked per core, >= 1)."""
    masked = np.flatnonzero(mask_b)
    unmasked = np.flatnonzero(~mask_b)
    counts = [0] * NCORES
    assign = [[] for _ in range(NCORES)]
    for i, s in enumerate(masked):
        k = i % NCORES
        assign[k].append(s)
        counts[k] += 1
    b = max(1, max(counts))
    # fill remaining slots with unmasked samples
    u = 0
    for k in range(NCORES):
        need = SHARD - counts[k]
        assign[k].extend(unmasked[u : u + need])
        u += need
    return b, [np.asarray(a, dtype=np.int64) for a in assign]


def make_in_maps(x, y, perm, mask, lam):
    """Host-side sharding: balanced assignment, masked-first ordering,
    and the cross-shard gather x[perm] for blend rows only."""
    x = np.ascontiguousarray(x, dtype=np.float32)
    y = np.ascontiguousarray(y, dtype=np.float32)
    perm = np.asarray(perm).astype(np.int64)
    mask_b = np.asarray(mask).astype(bool)
    lam = np.asarray(lam, dtype=np.float32)

    w = np.where(mask_b, lam, np.float32(1.0)).astype(np.float32)
    omw = (np.float32(1.0) - w).astype(np.float32)

    b, assign = _plan(mask_b)
    nf, nr = divmod(b, SPT)
    ncols = nf + nr

    in_maps = []
    for k in range(NCORES):
        sl = slice(k * SHARD, (k + 1) * SHARD)
        gidx = assign[k]
        w_r = w[gidx]
        omw_r = omw[gidx]
        psl = perm[gidx[:b]]                # partner rows for blend region only
        wc = np.empty((P, ncols), np.float32)
        owc = np.empty((P, ncols), np.float32)
        if nf:
            wc[:, :nf] = np.repeat(w_r[: nf * SPT].reshape(nf, SPT), PPS, axis=1).T
            owc[:, :nf] = np.repeat(
                omw_r[: nf * SPT].reshape(nf, SPT), PPS, axis=1
            ).T
        for j in range(nr):
            wc[:, nf + j] = w_r[nf * SPT + j]
            owc[:, nf + j] = omw_r[nf * SPT + j]
        wy = np.ascontiguousarray(np.stack([w[sl], omw[sl]], axis=1))
        in_maps.append(
            {
                "xs": np.ascontiguousarray(x[gidx].reshape(-1)),
                "xps": np.ascontiguousarray(x[psl].reshape(-1)),
                "wc": wc,
                "owc": owc,
                "ys": np.ascontiguousarray(y[sl]),
                "yps": np.ascontiguousarray(y[perm[sl]]),
                "wy": wy,
            }
        )
    return b, assign, in_maps


def assemble(results, assign):
    aug_x = np.empty((N, C, L), np.float32)
    aug_y = np.empty((N, NCLS), np.float32)
    for k in range(NCORES):
        sl = slice(k * SHARD, (k + 1) * SHARD)
        aug_x[assign[k]] = np.asarray(results[k]["ox"]).reshape(SHARD, C, L)
        aug_y[sl] = np.asarray(results[k]["oy"])
    return aug_x, aug_y


def kernel(x, y, perm, mask, lam):
    b, assign, in_maps = make_in_maps(x, y, perm, mask, lam)
    nc = get_nc(b)
    res = run_bass_kernel_spmd(nc, in_maps, core_ids=list(range(NCORES)))
    return assemble(res.results, assign)


# revision 8
# speedup vs baseline: 903.9314x; 903.9314x over previous
"""MixUp1D on Trainium2 (Bass/Tile), 8-core data-parallel.

Computes, for x:(N,C,L), y:(N,NC), perm/mask/lam:(N,):
    w        = where(mask, lam, 1.0)                  (host, O(N) scalars)
    aug_x[i] = w[i]*x[i] + (1-w[i])*x[perm[i]]
    aug_y[i] = w[i]*y[i] + (1-w[i])*y[perm[i]]

Sharding: N is split across 8 cores (32 samples each). perm indexes the
whole batch, so the cross-shard gather x[perm] is materialized on the
host while building the per-core inputs (the "all-to-all" realized at
shard time). Sample->core assignment is load-balanced: masked samples
are dealt round-robin so every core gets within +-1 of the mean.

Per-core device program (one NEFF per blend count B, cached):
  - First B samples (all masked ones, masked-first order): blend.
      4-sample tiles [128, 8192] (4 MB contiguous DMAs; 32 partitions
      per sample) + B%4 single-sample tiles [128, 2048], so the
      per-sample weight is a per-partition scalar:
        x_t  *= w                 (ACT Copy, per-partition scale, in place)
        xp_t  = xp_t*(1-w) + x_t  (fused scalar_tensor_tensor, in place)
  - Remaining samples have mask=False, so out == x bit-exactly
    (w=1 blend is 1*x + 0*xp): one contiguous DRAM->DRAM copy, skipping
    their x[perm] read entirely.
"""

import numpy as np

import concourse.bacc as bacc
import concourse.mybir as mybir
from concourse.tile import TileContext
from concourse.bass_utils import run_bass_kernel_spmd

# Problem shapes (fixed for this problem)
N, C, L = 256, 16, 16384
NCLS = 8
NCORES = 8
SHARD = N // NCORES          # 32 samples per core
ELEM = C * L                 # 262144 elements per sample
F = 8192                     # big-tile free dim (4 samples / 4 MB per tile)
FS = ELEM // 128             # single-sample tile free dim (2048)
P = 128                      # SBUF partitions
SPT = (P * F) // ELEM        # samples per big tile (4)
PPS = P // SPT               # partitions per sample in a big tile (32)

_CACHE: dict = {}


def _build_nc(b: int):
    """Build + compile the SPMD program blending the first `b` samples
    (1 <= b <= SHARD) and bulk-copying the rest."""
    assert 1 <= b <= SHARD
    nf, nr = divmod(b, SPT)      # full 4-sample tiles, single-sample tiles
    ncols = nf + nr              # weight columns
    f32 = mybir.dt.float32
    nc = bacc.Bacc(
        "TRN2",
        target_bir_lowering=False,
        debug=False,
        enable_asserts=False,
        num_devices=NCORES,
    )
    xs = nc.dram_tensor("xs", [SHARD * ELEM], f32, kind="ExternalInput")
    xps = nc.dram_tensor("xps", [b * ELEM], f32, kind="ExternalInput")
    wc = nc.dram_tensor("wc", [P, ncols], f32, kind="ExternalInput")
    owc = nc.dram_tensor("owc", [P, ncols], f32, kind="ExternalInput")
    ys = nc.dram_tensor("ys", [SHARD, NCLS], f32, kind="ExternalInput")
    yps = nc.dram_tensor("yps", [SHARD, NCLS], f32, kind="ExternalInput")
    wy = nc.dram_tensor("wy", [SHARD, 2], f32, kind="ExternalInput")
    ox = nc.dram_tensor("ox", [SHARD * ELEM], f32, kind="ExternalOutput")
    oy = nc.dram_tensor("oy", [SHARD, NCLS], f32, kind="ExternalOutput")

    Copy = mybir.ActivationFunctionType.Copy
    mult = mybir.AluOpType.mult
    add = mybir.AluOpType.add

    def blk(t1d, start_elems, p, f):
        return t1d[start_elems : start_elems + p * f].rearrange(
            "(p f) -> p f", p=p
        )

    with TileContext(nc) as tc:
        with (
            tc.tile_pool(name="const", bufs=1) as cp,
            tc.tile_pool(name="io", bufs=3) as io,
        ):
            wct = cp.tile([P, ncols], f32, tag="wct")
            owct = cp.tile([P, ncols], f32, tag="owct")
            nc.sync.dma_start(out=wct[:], in_=wc[:])
            nc.sync.dma_start(out=owct[:], in_=owc[:])

            # Unmasked tail: out == x, no x[perm] read needed. One
            # contiguous DRAM->DRAM copy on the SWDGE ring.
            if b < SHARD:
                nc.gpsimd.dma_start(out=ox[b * ELEM :], in_=xs[b * ELEM :])

            # 4-sample blend tiles
            for t in range(nf):
                off = t * SPT * ELEM
                xt = io.tile([P, F], f32, tag="x")
                xpt = io.tile([P, F], f32, tag="xp")
                nc.sync.dma_start(out=xt[:], in_=blk(xs, off, P, F))
                nc.sync.dma_start(out=xpt[:], in_=blk(xps, off, P, F))
                # x_t *= w (in-place on ScalarE)
                nc.scalar.activation(xt[:], xt[:], Copy, scale=wct[:, t : t + 1])
                # xp_t = (xp_t * (1-w)) + x_t (fused, in-place on VectorE)
                nc.vector.scalar_tensor_tensor(
                    xpt[:], xpt[:], owct[:, t : t + 1], xt[:], mult, add
                )
                # store from the ACT HWDGE ring so loads (SP ring) aren't
                # head-of-line blocked behind stores
                nc.scalar.dma_start(out=blk(ox, off, P, F), in_=xpt[:])

            # single-sample boundary blend tiles
            for j in range(nr):
                off = (nf * SPT + j) * ELEM
                c = nf + j
                xt = io.tile([P, FS], f32, tag="x")
                xpt = io.tile([P, FS], f32, tag="xp")
                nc.sync.dma_start(out=xt[:], in_=blk(xs, off, P, FS))
                nc.sync.dma_start(out=xpt[:], in_=blk(xps, off, P, FS))
                nc.scalar.activation(xt[:], xt[:], Copy, scale=wct[:, c : c + 1])
                nc.vector.scalar_tensor_tensor(
                    xpt[:], xpt[:], owct[:, c : c + 1], xt[:], mult, add
                )
                nc.scalar.dma_start(out=blk(ox, off, P, FS), in_=xpt[:])

            # y path: one tiny [32, 8] tile (full blend, original order)
            wyt = cp.tile([SHARD, 2], f32, tag="wyt")
            yt = cp.tile([SHARD, NCLS], f32, tag="yt")
            ypt = cp.tile([SHARD, NCLS], f32, tag="ypt")
            nc.sync.dma_start(out=wyt[:], in_=wy[:])
            nc.sync.dma_start(out=yt[:], in_=ys[:])
            nc.sync.dma_start(out=ypt[:], in_=yps[:])
            nc.scalar.activation(yt[:], yt[:], Copy, scale=wyt[:, 0:1])
            nc.vector.scalar_tensor_tensor(
                ypt[:], ypt[:], wyt[:, 1:2], yt[:], mult, add
            )
            nc.scalar.dma_start(out=oy[:], in_=ypt[:])

    nc.compile()
    return nc


def get_nc(b: int = SHARD):
    if b not in _CACHE:
        _CACHE[b] = _build_nc(b)
    return _CACHE[b]


def _plan(mask_b):
    """Load-balanced sample->core assignment, masked-first per core.

    Returns (b, assign) where assign[k] is the array of 32 global sample
    ids for core k (its masked samples first), and b is the per-core
    blend count (max masked per core, >= 1)."""
    masked = np.flatnonzero(mask_b)
    unmasked = np.flatnonzero(~mask_b)
    counts = [0] * NCORES
    assign = [[] for _ in range(NCORES)]
    for i, s in enumerate(masked):
        k = i % NCORES
        assign[k].append(s)
        counts[k] += 1
    b = max(1, max(counts))
    # fill remaining slots with unmasked samples
    u = 0
    for k in range(NCORES):
        need = SHARD - counts[k]
        assign[k].extend(unmasked[u : u + need])
        u += need
    return b, [np.asarray(a, dtype=np.int64) for a in assign]


def make_in_maps(x, y, perm, mask, lam):
    """Host-side sharding: balanced assignment, masked-first ordering,
    and the cross-shard gather x[perm] for blend rows only."""
    x = np.ascontiguousarray(x, dtype=np.float32)
    y = np.ascontiguousarray(y, dtype=np.float32)
    perm = np.asarray(perm).astype(np.int64)
    # match jax gather semantics for out-of-range indices (wrap
    # negatives once, clamp the rest); no-op for valid permutations
    perm = np.clip(np.where(perm < 0, perm + N, perm), 0, N - 1)
    mask_b = np.asarray(mask).astype(bool)
    lam = np.asarray(lam, dtype=np.float32)

    w = np.where(mask_b, lam, np.float32(1.0)).astype(np.float32)
    omw = (np.float32(1.0) - w).astype(np.float32)

    b, assign = _plan(mask_b)
    nf, nr = divmod(b, SPT)
    ncols = nf + nr

    in_maps = []
    for k in range(NCORES):
        sl = slice(k * SHARD, (k + 1) * SHARD)
        gidx = assign[k]
        w_r = w[gidx]
        omw_r = omw[gidx]
        psl = perm[gidx[:b]]                # partner rows for blend region only
        wc = np.empty((P, ncols), np.float32)
        owc = np.empty((P, ncols), np.float32)
        if nf:
            wc[:, :nf] = np.repeat(w_r[: nf * SPT].reshape(nf, SPT), PPS, axis=1).T
            owc[:, :nf] = np.repeat(
                omw_r[: nf * SPT].reshape(nf, SPT), PPS, axis=1
            ).T
        for j in range(nr):
            wc[:, nf + j] = w_r[nf * SPT + j]
            owc[:, nf + j] = omw_r[nf * SPT + j]
        wy = np.ascontiguousarray(np.stack([w[sl], omw[sl]], axis=1))
        in_maps.append(
            {
                "xs": np.ascontiguousarray(x[gidx].reshape(-1)),
                "xps": np.ascontiguousarray(x[psl].reshape(-1)),
                "wc": wc,
                "owc": owc,
                "ys": np.ascontiguousarray(y[sl]),
                "yps": np.ascontiguousarray(y[perm[sl]]),
                "wy": wy,
            }
        )
    return b, assign, in_maps


def assemble(results, assign):
    aug_x = np.empty((N, C, L), np.float32)
    aug_y = np.empty((N, NCLS), np.float32)
    for k in range(NCORES):
        sl = slice(k * SHARD, (k + 1) * SHARD)
        aug_x[assign[k]] = np.asarray(results[k]["ox"]).reshape(SHARD, C, L)
        aug_y[sl] = np.asarray(results[k]["oy"])
    return aug_x, aug_y


def kernel(x, y, perm, mask, lam):
    b, assign, in_maps = make_in_maps(x, y, perm, mask, lam)
    nc = get_nc(b)
    res = run_bass_kernel_spmd(nc, in_maps, core_ids=list(range(NCORES)))
    return assemble(res.results, assign)
